# revision 10
# baseline (speedup 1.0000x reference)
"""Chamfer distance kernel for Trainium2 (8 NeuronCores).

Problem: B=8 batches of two 3-D point clouds x1,x2 of 8192 points each.
  D[b,n,m] = ||x1[b,n] - x2[b,m]||^2   (clamped at 0)
  dist1[b,n] = min_m D,  dist2[b,m] = min_n D,
  loss = mean(dist1) + mean(dist2)

Sharding (per hint): data-parallel over batch -> 1 batch per core; within a
core, tile over query chunks of 128.

Device algorithm per core:
  G = -D is produced directly by the TensorE via the augmented K=5 identity
      -D[n,m] = sum_k P[k,n] Q[k,m],
      P = -[x1x, x1y, x1z, ||x1||^2, 1],  Q = [-2x2x, -2x2y, -2x2z, 1, ||x2||^2]
  A [5,128] weight chunk x [5,512] moving tile -> one [128,512] PSUM bank of G.
  fp32 matmuls are issued into the 4 PE row groups (tile_position=(32g,0),
  weights replicated at partitions 0/32/64/96): measured 214 ns per fp32
  [5,128]x[5,512] matmul on this part, vs 895 ns without row-grouping.
  The DVE then max-reduces each 4-bank PSUM tile (dist1 = -max G); DVE is
  the bottleneck engine at 1 element/lane/cycle (no faster max path exists
  on this part: TTR ucode is mult/add-only, tensor_scalar accum runs 1x,
  GpSimd has no elementwise ISA, ScalarE cannot reduce).
  Pass 2 swaps the roles of P and Q to get dist2.  Host negates, clamps,
  reorders and computes the scalar loss.
"""

import os
import sys
from contextlib import ExitStack

sys.path.insert(0, "/root/shim")
for _p in ("/opt/trn_rl_repo", "/root/.axon_site", "/root/.axon_site/_ro/pypackages"):
    if os.path.isdir(_p) and _p not in sys.path:
        sys.path.append(_p)

import numpy as np

_B, _N, _M = 8, 8192, 8192
_K = 5            # augmented contraction dim
_PC = 128         # output partition chunk (points per weight load)
_FD = 512         # matmul free dim (one fp32 PSUM bank)
_PS_COLS = 2048   # psum tile free size (4 banks)
_GC = 4           # chunks per combine group

_cache = {}


def _aug_arrays(input1, input2):
    """Host-side layout prep: P' (negated aug of x1) and Q (aug of x2),
    both [B, 5, N] fp32."""
    x1 = np.asarray(input1, dtype=np.float32)
    x2 = np.asarray(input2, dtype=np.float32)
    B, N, _ = x1.shape
    M = x2.shape[1]
    n1 = np.sum(x1 * x1, axis=2)
    n2 = np.sum(x2 * x2, axis=2)
    P = np.empty((B, _K, N), dtype=np.float32)
    P[:, 0:3, :] = -np.transpose(x1, (0, 2, 1))
    P[:, 3, :] = -n1
    P[:, 4, :] = -1.0
    Q = np.empty((B, _K, M), dtype=np.float32)
    Q[:, 0:3, :] = -2.0 * np.transpose(x2, (0, 2, 1))
    Q[:, 3, :] = 1.0
    Q[:, 4, :] = n2
    return P, Q


def _build_program(n, m):
    import concourse.bacc as bacc
    import concourse.mybir as mybir
    import concourse.tile as tile

    f32 = mybir.dt.float32
    nc = bacc.Bacc("TRN2", target_bir_lowering=False, debug=False, num_devices=8)

    p_dram = nc.dram_tensor("p_aug", [_K, n], f32, kind="ExternalInput")
    q_dram = nc.dram_tensor("q_aug", [_K, m], f32, kind="ExternalInput")
    g1_dram = nc.dram_tensor("g1", [_PC, n // _PC], f32, kind="ExternalOutput")
    g2_dram = nc.dram_tensor("g2", [_PC, m // _PC], f32, kind="ExternalOutput")

    ps_cols = min(_PS_COLS, m)

    with tile.TileContext(nc) as tc, ExitStack() as ctx:
        const_pool = ctx.enter_context(tc.tile_pool(name="const", bufs=1))
        psum_pool = ctx.enter_context(tc.tile_pool(name="psum", bufs=2, space="PSUM"))
        comb_pool = ctx.enter_context(tc.tile_pool(name="comb", bufs=4))
        res_pool = ctx.enter_context(tc.tile_pool(name="res", bufs=1))

        # Weights/streams replicated into the 4 PE row groups (partitions
        # 0,32,64,96) so fp32 matmuls can use per-row-group tile_position.
        p_sb = const_pool.tile([96 + _K, n], f32, name="p_sb")
        q_sb = const_pool.tile([96 + _K, m], f32, name="q_sb")
        for g in range(4):
            nc.sync.dma_start(p_sb[32 * g:32 * g + _K, :], p_dram[:])
            nc.sync.dma_start(q_sb[32 * g:32 * g + _K, :], q_dram[:])

        res1 = res_pool.tile([_PC, n // _PC], f32, name="res1")
        res2 = res_pool.tile([_PC, m // _PC], f32, name="res2")

        def one_pass(w_sb, s_sb, res, n_pts, m_pts):
            n_chunks = n_pts // _PC
            rounds = m_pts // ps_cols
            mm_per_round = ps_cols // _FD
            assert n_chunks % _GC == 0
            for cg in range(n_chunks // _GC):
                comb = comb_pool.tile([_PC, _GC * rounds * 2], f32, name="comb",
                                      tag="comb")
                for ci in range(_GC):
                    c = cg * _GC + ci
                    for r in range(rounds):
                        ps = psum_pool.tile([_PC, ps_cols], f32, name="ps",
                                            tag="ps")
                        for j in range(mm_per_round):
                            g = j % 4
                            m0 = r * ps_cols + j * _FD
                            nc.tensor.matmul(
                                ps[:, j * _FD:(j + 1) * _FD],
                                w_sb[32 * g:32 * g + _K, c * _PC:(c + 1) * _PC],
                                s_sb[32 * g:32 * g + _K, m0:m0 + _FD],
                                start=True, stop=True,
                                tile_position=(32 * g, 0),
                            )
                        for h in range(2):
                            hw = ps_cols // 2
                            nc.vector.tensor_reduce(
                                comb[:, (ci * rounds + r) * 2 + h:
                                        (ci * rounds + r) * 2 + h + 1],
                                ps[:, h * hw:(h + 1) * hw],
                                axis=mybir.AxisListType.X,
                                op=mybir.AluOpType.max,
                            )
                # one combine per group: [P, GC, rounds] -max-> [P, GC]
                nc.vector.tensor_reduce(
                    res[:, cg * _GC:(cg + 1) * _GC],
                    comb[:].rearrange("p (c r) -> p c r", r=rounds * 2),
                    axis=mybir.AxisListType.X,
                    op=mybir.AluOpType.max,
                )

        one_pass(p_sb, q_sb, res1, n, m)
        one_pass(q_sb, p_sb, res2, m, n)

        nc.sync.dma_start(g1_dram[:], res1[:])
        nc.sync.dma_start(g2_dram[:], res2[:])

    nc.compile()
    return nc


def _get_program(n, m):
    key = (n, m)
    if key not in _cache:
        _cache[key] = _build_program(n, m)
    return _cache[key]


def kernel(input1, input2):
    from concourse import bass_utils

    x1 = np.asarray(input1, dtype=np.float32)
    x2 = np.asarray(input2, dtype=np.float32)
    B, N, _ = x1.shape
    M = x2.shape[1]

    P, Q = _aug_arrays(x1, x2)
    nc = _get_program(N, M)

    in_maps = [{"p_aug": np.ascontiguousarray(P[b]),
                "q_aug": np.ascontiguousarray(Q[b])} for b in range(B)]
    res = bass_utils.run_bass_kernel_spmd(nc, in_maps, list(range(B)))

    dist1 = np.empty((B, N), dtype=np.float32)
    dist2 = np.empty((B, M), dtype=np.float32)
    for b in range(B):
        g1 = res.results[b]["g1"]  # [128, N//128]; point n = c*128+p -> g1[p, c]
        g2 = res.results[b]["g2"]
        dist1[b] = np.maximum(-g1.T.reshape(-1), 0.0)
        dist2[b] = np.maximum(-g2.T.reshape(-1), 0.0)

    loss = np.float32(dist1.mean(dtype=np.float64) + dist2.mean(dtype=np.float64))
    return (loss, dist1, dist2)


# revision 11
# speedup vs baseline: 1.0043x; 1.0043x over previous
"""Chamfer distance kernel for Trainium2 (8 NeuronCores).

Problem: B=8 batches of two 3-D point clouds x1,x2 of 8192 points each.
  D[b,n,m] = ||x1[b,n] - x2[b,m]||^2   (clamped at 0)
  dist1[b,n] = min_m D,  dist2[b,m] = min_n D,
  loss = mean(dist1) + mean(dist2)

Sharding (per hint): data-parallel over batch -> 1 batch per core; within a
core, tile over query chunks of 128.

Device algorithm per core:
  G = -D is produced directly by the TensorE via the augmented K=5 identity
      -D[n,m] = sum_k P[k,n] Q[k,m],
      P = -[x1x, x1y, x1z, ||x1||^2, 1],  Q = [-2x2x, -2x2y, -2x2z, 1, ||x2||^2]
  A [5,128] weight chunk x [5,512] moving tile -> one [128,512] PSUM bank of G.
  fp32 matmuls are issued into the 4 PE row groups (tile_position=(32g,0),
  weights replicated at partitions 0/32/64/96): measured 214 ns per fp32
  [5,128]x[5,512] matmul on this part, vs 895 ns without row-grouping.
  The DVE then max-reduces each 4-bank PSUM tile (dist1 = -max G); DVE is
  the bottleneck engine at 1 element/lane/cycle (no faster max path exists
  on this part: TTR ucode is mult/add-only, tensor_scalar accum runs 1x,
  GpSimd has no elementwise ISA, ScalarE cannot reduce).
  Pass 2 swaps the roles of P and Q to get dist2.  Host negates, clamps,
  reorders and computes the scalar loss.
"""

import os
import sys
from contextlib import ExitStack

sys.path.insert(0, "/root/shim")
for _p in ("/opt/trn_rl_repo", "/root/.axon_site", "/root/.axon_site/_ro/pypackages"):
    if os.path.isdir(_p) and _p not in sys.path:
        sys.path.append(_p)

import numpy as np

_B, _N, _M = 8, 8192, 8192
_K = 5            # augmented contraction dim
_PC = 128         # output partition chunk (points per weight load)
_FD = 512         # matmul free dim (one fp32 PSUM bank)
_PS_COLS = 2048   # psum tile free size (4 banks)
_GC = 4           # chunks per combine group

_cache = {}


def _aug_arrays(input1, input2):
    """Host-side layout prep: P' (negated aug of x1) and Q (aug of x2),
    both [B, 5, N] fp32."""
    x1 = np.asarray(input1, dtype=np.float32)
    x2 = np.asarray(input2, dtype=np.float32)
    B, N, _ = x1.shape
    M = x2.shape[1]
    n1 = np.sum(x1 * x1, axis=2)
    n2 = np.sum(x2 * x2, axis=2)
    P = np.empty((B, _K, N), dtype=np.float32)
    P[:, 0:3, :] = -np.transpose(x1, (0, 2, 1))
    P[:, 3, :] = -n1
    P[:, 4, :] = -1.0
    Q = np.empty((B, _K, M), dtype=np.float32)
    Q[:, 0:3, :] = -2.0 * np.transpose(x2, (0, 2, 1))
    Q[:, 3, :] = 1.0
    Q[:, 4, :] = n2
    return P, Q


def _build_program(n, m):
    import concourse.bacc as bacc
    import concourse.mybir as mybir
    import concourse.tile as tile

    f32 = mybir.dt.float32
    nc = bacc.Bacc("TRN2", target_bir_lowering=False, debug=False, num_devices=8)

    p_dram = nc.dram_tensor("p_aug", [_K, n], f32, kind="ExternalInput")
    q_dram = nc.dram_tensor("q_aug", [_K, m], f32, kind="ExternalInput")
    g1_dram = nc.dram_tensor("g1", [_PC, n // _PC], f32, kind="ExternalOutput")
    g2_dram = nc.dram_tensor("g2", [_PC, m // _PC], f32, kind="ExternalOutput")

    ps_cols = min(_PS_COLS, m)

    with tile.TileContext(nc) as tc, ExitStack() as ctx:
        const_pool = ctx.enter_context(tc.tile_pool(name="const", bufs=1))
        psum_pool = ctx.enter_context(tc.tile_pool(name="psum", bufs=2, space="PSUM"))
        comb_pool = ctx.enter_context(tc.tile_pool(name="comb", bufs=4))
        res_pool = ctx.enter_context(tc.tile_pool(name="res", bufs=1))

        # Weights/streams replicated into the 4 PE row groups (partitions
        # 0,32,64,96) so fp32 matmuls can use per-row-group tile_position.
        # Chunked DMAs across both HWDGE queues: a single [5, n] dma_start
        # lands on one DMA engine (~18 GB/s) and serializes the startup.
        p_sb = const_pool.tile([96 + _K, n], f32, name="p_sb")
        q_sb = const_pool.tile([96 + _K, m], f32, name="q_sb")
        n_dma = 4
        for g in range(4):
            for d in range(n_dma):
                c0, c1 = d * n // n_dma, (d + 1) * n // n_dma
                nc.sync.dma_start(p_sb[32 * g:32 * g + _K, c0:c1],
                                  p_dram[:, c0:c1])
                c0, c1 = d * m // n_dma, (d + 1) * m // n_dma
                nc.scalar.dma_start(q_sb[32 * g:32 * g + _K, c0:c1],
                                    q_dram[:, c0:c1])

        res1 = res_pool.tile([_PC, n // _PC], f32, name="res1")
        res2 = res_pool.tile([_PC, m // _PC], f32, name="res2")

        def one_pass(w_sb, s_sb, res, n_pts, m_pts):
            n_chunks = n_pts // _PC
            rounds = m_pts // ps_cols
            mm_per_round = ps_cols // _FD
            assert n_chunks % _GC == 0
            for cg in range(n_chunks // _GC):
                comb = comb_pool.tile([_PC, _GC * rounds * 2], f32, name="comb",
                                      tag="comb")
                for ci in range(_GC):
                    c = cg * _GC + ci
                    for r in range(rounds):
                        ps = psum_pool.tile([_PC, ps_cols], f32, name="ps",
                                            tag="ps")
                        for j in range(mm_per_round):
                            g = j % 4
                            m0 = r * ps_cols + j * _FD
                            nc.tensor.matmul(
                                ps[:, j * _FD:(j + 1) * _FD],
                                w_sb[32 * g:32 * g + _K, c * _PC:(c + 1) * _PC],
                                s_sb[32 * g:32 * g + _K, m0:m0 + _FD],
                                start=True, stop=True,
                                tile_position=(32 * g, 0),
                            )
                        for h in range(2):
                            hw = ps_cols // 2
                            nc.vector.tensor_reduce(
                                comb[:, (ci * rounds + r) * 2 + h:
                                        (ci * rounds + r) * 2 + h + 1],
                                ps[:, h * hw:(h + 1) * hw],
                                axis=mybir.AxisListType.X,
                                op=mybir.AluOpType.max,
                            )
                # one combine per group: [P, GC, rounds] -max-> [P, GC]
                nc.vector.tensor_reduce(
                    res[:, cg * _GC:(cg + 1) * _GC],
                    comb[:].rearrange("p (c r) -> p c r", r=rounds * 2),
                    axis=mybir.AxisListType.X,
                    op=mybir.AluOpType.max,
                )

        one_pass(p_sb, q_sb, res1, n, m)
        one_pass(q_sb, p_sb, res2, m, n)

        nc.sync.dma_start(g1_dram[:], res1[:])
        nc.sync.dma_start(g2_dram[:], res2[:])

    nc.compile()
    return nc


def _get_program(n, m):
    key = (n, m)
    if key not in _cache:
        _cache[key] = _build_program(n, m)
    return _cache[key]


def kernel(input1, input2):
    from concourse import bass_utils

    x1 = np.asarray(input1, dtype=np.float32)
    x2 = np.asarray(input2, dtype=np.float32)
    B, N, _ = x1.shape
    M = x2.shape[1]

    P, Q = _aug_arrays(x1, x2)
    nc = _get_program(N, M)

    in_maps = [{"p_aug": np.ascontiguousarray(P[b]),
                "q_aug": np.ascontiguousarray(Q[b])} for b in range(B)]
    res = bass_utils.run_bass_kernel_spmd(nc, in_maps, list(range(B)))

    dist1 = np.empty((B, N), dtype=np.float32)
    dist2 = np.empty((B, M), dtype=np.float32)
    for b in range(B):
        g1 = res.results[b]["g1"]  # [128, N//128]; point n = c*128+p -> g1[p, c]
        g2 = res.results[b]["g2"]
        dist1[b] = np.maximum(-g1.T.reshape(-1), 0.0)
        dist2[b] = np.maximum(-g2.T.reshape(-1), 0.0)

    loss = np.float32(dist1.mean(dtype=np.float64) + dist2.mean(dtype=np.float64))
    return (loss, dist1, dist2)


# revision 12
# speedup vs baseline: 1.0123x; 1.0080x over previous
"""Chamfer distance kernel for Trainium2 (8 NeuronCores).

Problem: B=8 batches of two 3-D point clouds x1,x2 of 8192 points each.
  D[b,n,m] = ||x1[b,n] - x2[b,m]||^2   (clamped at 0)
  dist1[b,n] = min_m D,  dist2[b,m] = min_n D,
  loss = mean(dist1) + mean(dist2)

Sharding (per hint): data-parallel over batch -> 1 batch per core; within a
core, tile over query chunks of 128.

Device algorithm per core:
  G = -D is produced directly by the TensorE via the augmented K=5 identity
      -D[n,m] = sum_k P[k,n] Q[k,m],
      P = -[x1x, x1y, x1z, ||x1||^2, 1],  Q = [-2x2x, -2x2y, -2x2z, 1, ||x2||^2]
  A [5,128] weight chunk x [5,512] moving tile -> one [128,512] PSUM bank of G.
  fp32 matmuls are issued into the 4 PE row groups (tile_position=(32g,0),
  weights replicated at partitions 0/32/64/96): measured 214 ns per fp32
  [5,128]x[5,512] matmul on this part, vs 895 ns without row-grouping.
  The DVE then max-reduces each 4-bank PSUM tile (dist1 = -max G); DVE is
  the bottleneck engine at 1 element/lane/cycle (no faster max path exists
  on this part: TTR ucode is mult/add-only, tensor_scalar accum runs 1x,
  GpSimd has no elementwise ISA, ScalarE cannot reduce).
  Pass 2 swaps the roles of P and Q to get dist2.  Host negates, clamps,
  reorders and computes the scalar loss.
"""

import os
import sys
from contextlib import ExitStack

sys.path.insert(0, "/root/shim")
for _p in ("/opt/trn_rl_repo", "/root/.axon_site", "/root/.axon_site/_ro/pypackages"):
    if os.path.isdir(_p) and _p not in sys.path:
        sys.path.append(_p)

import numpy as np

_B, _N, _M = 8, 8192, 8192
_K = 5            # augmented contraction dim
_PC = 128         # output partition chunk (points per weight load)
_FD = 512         # matmul free dim (one fp32 PSUM bank)
_PS_COLS = 2048   # psum tile free size (4 banks)
_GC = 4           # chunks per combine group

_cache = {}


def _aug_arrays(input1, input2):
    """Host-side layout prep: P' (negated aug of x1) and Q (aug of x2),
    both [B, 5, N] fp32."""
    x1 = np.asarray(input1, dtype=np.float32)
    x2 = np.asarray(input2, dtype=np.float32)
    B, N, _ = x1.shape
    M = x2.shape[1]
    n1 = np.sum(x1 * x1, axis=2)
    n2 = np.sum(x2 * x2, axis=2)
    P = np.empty((B, _K, N), dtype=np.float32)
    P[:, 0:3, :] = -np.transpose(x1, (0, 2, 1))
    P[:, 3, :] = -n1
    P[:, 4, :] = -1.0
    Q = np.empty((B, _K, M), dtype=np.float32)
    Q[:, 0:3, :] = -2.0 * np.transpose(x2, (0, 2, 1))
    Q[:, 3, :] = 1.0
    Q[:, 4, :] = n2
    return P, Q


def _build_program(n, m):
    import concourse.bacc as bacc
    import concourse.mybir as mybir
    import concourse.tile as tile

    f32 = mybir.dt.float32
    nc = bacc.Bacc("TRN2", target_bir_lowering=False, debug=False, num_devices=8)

    p_dram = nc.dram_tensor("p_aug", [_K, n], f32, kind="ExternalInput")
    q_dram = nc.dram_tensor("q_aug", [_K, m], f32, kind="ExternalInput")
    g1_dram = nc.dram_tensor("g1", [_PC, n // _PC], f32, kind="ExternalOutput")
    g2_dram = nc.dram_tensor("g2", [_PC, m // _PC], f32, kind="ExternalOutput")

    ps_cols = min(_PS_COLS, m)

    with tile.TileContext(nc) as tc, ExitStack() as ctx:
        const_pool = ctx.enter_context(tc.tile_pool(name="const", bufs=1))
        psum_pool = ctx.enter_context(tc.tile_pool(name="psum", bufs=2, space="PSUM"))
        comb_pool = ctx.enter_context(tc.tile_pool(name="comb", bufs=4))
        res_pool = ctx.enter_context(tc.tile_pool(name="res", bufs=1))

        # Weights/streams replicated into the 4 PE row groups (partitions
        # 0,32,64,96) so fp32 matmuls can use per-row-group tile_position.
        # Chunked DMAs across both HWDGE queues: a single [5, n] dma_start
        # lands on one DMA engine (~18 GB/s) and serializes the startup.
        p_sb = const_pool.tile([96 + _K, n], f32, name="p_sb")
        q_sb = const_pool.tile([96 + _K, m], f32, name="q_sb")
        n_dma = 4
        for d in range(n_dma):
            for g in range(4):
                c0, c1 = d * n // n_dma, (d + 1) * n // n_dma
                nc.sync.dma_start(p_sb[32 * g:32 * g + _K, c0:c1],
                                  p_dram[:, c0:c1])
                c0, c1 = d * m // n_dma, (d + 1) * m // n_dma
                nc.scalar.dma_start(q_sb[32 * g:32 * g + _K, c0:c1],
                                    q_dram[:, c0:c1])

        res1 = res_pool.tile([_PC, n // _PC], f32, name="res1")
        res2 = res_pool.tile([_PC, m // _PC], f32, name="res2")

        def one_pass(w_sb, s_sb, res, n_pts, m_pts):
            n_chunks = n_pts // _PC
            rounds = m_pts // ps_cols
            mm_per_round = ps_cols // _FD
            assert n_chunks % _GC == 0
            for cg in range(n_chunks // _GC):
                comb = comb_pool.tile([_PC, _GC * rounds * 2], f32, name="comb",
                                      tag="comb")
                for ci in range(_GC):
                    c = cg * _GC + ci
                    for r in range(rounds):
                        ps = psum_pool.tile([_PC, ps_cols], f32, name="ps",
                                            tag="ps")
                        for j in range(mm_per_round):
                            g = j % 4
                            m0 = r * ps_cols + j * _FD
                            nc.tensor.matmul(
                                ps[:, j * _FD:(j + 1) * _FD],
                                w_sb[32 * g:32 * g + _K, c * _PC:(c + 1) * _PC],
                                s_sb[32 * g:32 * g + _K, m0:m0 + _FD],
                                start=True, stop=True,
                                tile_position=(32 * g, 0),
                            )
                        for h in range(2):
                            hw = ps_cols // 2
                            nc.vector.tensor_reduce(
                                comb[:, (ci * rounds + r) * 2 + h:
                                        (ci * rounds + r) * 2 + h + 1],
                                ps[:, h * hw:(h + 1) * hw],
                                axis=mybir.AxisListType.X,
                                op=mybir.AluOpType.max,
                            )
                # one combine per group: [P, GC, rounds] -max-> [P, GC]
                nc.vector.tensor_reduce(
                    res[:, cg * _GC:(cg + 1) * _GC],
                    comb[:].rearrange("p (c r) -> p c r", r=rounds * 2),
                    axis=mybir.AxisListType.X,
                    op=mybir.AluOpType.max,
                )

        one_pass(p_sb, q_sb, res1, n, m)
        one_pass(q_sb, p_sb, res2, m, n)

        nc.sync.dma_start(g1_dram[:], res1[:])
        nc.sync.dma_start(g2_dram[:], res2[:])

    nc.compile()
    return nc


def _get_program(n, m):
    key = (n, m)
    if key not in _cache:
        _cache[key] = _build_program(n, m)
    return _cache[key]


def kernel(input1, input2):
    from concourse import bass_utils

    x1 = np.asarray(input1, dtype=np.float32)
    x2 = np.asarray(input2, dtype=np.float32)
    B, N, _ = x1.shape
    M = x2.shape[1]

    P, Q = _aug_arrays(x1, x2)
    nc = _get_program(N, M)

    in_maps = [{"p_aug": np.ascontiguousarray(P[b]),
                "q_aug": np.ascontiguousarray(Q[b])} for b in range(B)]
    res = bass_utils.run_bass_kernel_spmd(nc, in_maps, list(range(B)))

    dist1 = np.empty((B, N), dtype=np.float32)
    dist2 = np.empty((B, M), dtype=np.float32)
    for b in range(B):
        g1 = res.results[b]["g1"]  # [128, N//128]; point n = c*128+p -> g1[p, c]
        g2 = res.results[b]["g2"]
        dist1[b] = np.maximum(-g1.T.reshape(-1), 0.0)
        dist2[b] = np.maximum(-g2.T.reshape(-1), 0.0)

    loss = np.float32(dist1.mean(dtype=np.float64) + dist2.mean(dtype=np.float64))
    return (loss, dist1, dist2)
